# revision 47
# baseline (speedup 1.0000x reference)
"""MLA-style attention (nn_Attention_15496242004691) on 8 trn2 NeuronCores.

Strategy (v3):
  Launch 1 (token-sharded, 512 tokens/core): A projections (bf16 matmuls,
    fp32 PSUM), RMSNorm (norm weights folded into the B projections on
    host), RoPE of k_pe (pair-swap folded into an extended wkv_a on host),
    plus the token-shardable halves of the B projections: k_nope^T (fp8)
    and v (bf16) for ALL heads. Emits cqn (bf16), knT/kpe (fp8), v (bf16).
  Launch 2 (head-sharded, 2 heads/core): q B-projection + q RoPE into a
    paired fp8 layout, causal attention with transposed scores
    ([k, q] tiles), output projection; host sums 8 bf16 partials in fp32.

Score matmuls run as fp8e4m3 DoubleRow (contraction pairs
[nope(128); rope(64)+zeros], 0.5 PE cycles/row); everything else is bf16
(1 cycle/row, half the DMA/SBUF traffic of fp32r). Softmax denominators:
exp tiles pair-summed on DVE, then ones-column matmuls into a PSUM row.
Elementwise work is balanced across ACT (exp + casts) and DVE (masks,
denominator pairs, normalize, casts); attention is software-pipelined 3
score tiles deep with wo chunks interleaved to fill PE bubbles. DMAs are
batched into few large descriptors (HWDGE serializes per instruction).
"""

import numpy as np
import ml_dtypes

import concourse.bass as bass
import concourse.mybir as mybir
import concourse.tile as tile
from concourse import bacc
from concourse.bass_utils import run_bass_kernel_spmd

F32 = mybir.dt.float32
F32R = mybir.dt.float32r
BF16 = mybir.dt.bfloat16
F8 = mybir.dt.float8e4
AF = mybir.ActivationFunctionType

NPBF = ml_dtypes.bfloat16
NPF8 = ml_dtypes.float8_e4m3

B, S, DIM, H = 2, 2048, 2048, 16
NCORES = 8
HPC = H // NCORES  # heads per core = 2
RQ = RKV = 512
DN, DR, DV, DQK = 128, 64, 128, 192
EPS = 1e-6
SCALE = DQK ** -0.5
T = B * S          # 4096 tokens
TS = T // NCORES   # 512 tokens per core in launch 1

_CACHE = {}


# --------------------------------------------------------------------------
# Launch 1: A-projections + RMSNorm + k_pe RoPE (token-sharded)
# --------------------------------------------------------------------------
def build_k1():
    nc = bacc.Bacc("TRN2", target_bir_lowering=False)
    xt = nc.dram_tensor("xt", [DIM, TS], BF16, kind="ExternalInput")
    wqa = nc.dram_tensor("wqa", [128, 16, RQ], BF16, kind="ExternalInput")
    wkva = nc.dram_tensor("wkva", [128, 16, RKV], BF16, kind="ExternalInput")
    wkvap = nc.dram_tensor("wkvap", [128, 16, 2 * DR], BF16, kind="ExternalInput")
    cosk = nc.dram_tensor("cosk", [DR, TS], F32, kind="ExternalInput")
    sink = nc.dram_tensor("sink", [DR, TS], F32, kind="ExternalInput")
    onec = nc.dram_tensor("onec", [128, 1], BF16, kind="ExternalInput")
    oner = nc.dram_tensor("oner", [1, 128], F32R, kind="ExternalInput")
    wkbf = nc.dram_tensor("wkbf", [128, 4, H * DN], BF16, kind="ExternalInput")
    wvbf = nc.dram_tensor("wvbf", [128, 4, H * DV], BF16, kind="ExternalInput")
    cqn = nc.dram_tensor("cqn", [RQ, TS], BF16, kind="ExternalOutput")
    knT = nc.dram_tensor("knT", [H * DN, TS], F8, kind="ExternalOutput")
    vout = nc.dram_tensor("vout", [TS, H * DV], BF16, kind="ExternalOutput")
    kpe = nc.dram_tensor("kpe", [DR, TS], F8, kind="ExternalOutput")

    with tile.TileContext(nc) as tc:
        with tc.tile_pool(name="const", bufs=1) as cpool, \
             tc.tile_pool(name="sb", bufs=2) as sb, \
             tc.tile_pool(name="ps", bufs=1, space="PSUM") as ps:
            ones_col = cpool.tile([128, 1], BF16)
            ones_row = cpool.tile([1, 128], F32R)
            eps_t = cpool.tile([1, 1], F32)
            nc.vector.memset(eps_t, EPS)

            xt_t = cpool.tile([128, 16, TS], BF16)
            xt_r = xt[:, :].rearrange("(k p) t -> p k t", p=128)
            wqa_t = cpool.tile([128, 16, RQ], BF16)
            wkva_t = cpool.tile([128, 16, RKV], BF16)
            wkvap_t = cpool.tile([128, 16, 2 * DR], BF16)
            cos_t = cpool.tile([DR, TS], F32)
            sin_t = cpool.tile([DR, TS], F32)
            # few big DMAs (HWDGE serializes per-instruction); graduated
            # chunks so the first matmuls can start early
            chunks = [(0, 1), (1, 2), (2, 4), (4, 8), (8, 16)]
            for lo, hi in chunks:
                ksl = slice(lo, hi)
                nc.sync.dma_start(out=wqa_t[:, ksl, :], in_=wqa[:, ksl, :])
                nc.sync.dma_start(out=xt_t[:, ksl, :], in_=xt_r[:, ksl, :])
                if hi == 4:
                    # pe columns early: the kv pe tile runs right after q
                    nc.sync.dma_start(out=wkvap_t, in_=wkvap[:, :, :])
            for c2 in range(2):
                ksl = slice(8 * c2, 8 * c2 + 8)
                nc.sync.dma_start(out=wkva_t[:, ksl, :], in_=wkva[:, ksl, :])
            nc.sync.dma_start(out=cos_t, in_=cosk[:, :])
            nc.sync.dma_start(out=sin_t, in_=sink[:, :])
            nc.sync.dma_start(out=ones_col, in_=onec[:, :])
            nc.sync.dma_start(out=ones_row, in_=oner[:, :])
            wkb_t = cpool.tile([128, 4, H * DN], BF16)
            wvb_t = cpool.tile([128, 4, H * DV], BF16)
            for c2 in range(2):
                csl = slice(H * DN // 2 * c2, H * DN // 2 * (c2 + 1))
                nc.sync.dma_start(out=wkb_t[:, :, csl], in_=wkbf[:, :, csl])
                nc.sync.dma_start(out=wvb_t[:, :, csl], in_=wvbf[:, :, csl])

            # q path: k-outer (matches the DMA chunk arrival order)
            q_accs = []
            for m in range(4):
                acc = ps.tile([128, TS], F32, tag=f"mm{m}", bufs=1, name="acc")
                q_accs.append(acc)
            for k in range(16):
                for m in range(4):
                    nc.tensor.matmul(q_accs[m], wqa_t[:, k, m * 128:(m + 1) * 128],
                                     xt_t[:, k, :], start=(k == 0), stop=(k == 15))
            q_sqs = []
            for m in range(4):
                sq = sb.tile([128, TS], BF16, tag=f"sq{m}", bufs=1, name="sq")
                nc.scalar.activation(sq, q_accs[m], AF.Square)
                q_sqs.append(sq)

            # kv pe tile: its matmuls hide the q Square/rsqrt chain
            pe = ps.tile([128, TS], F32, tag="mm4", bufs=1, name="pe")
            for k in range(16):
                nc.tensor.matmul(pe, wkvap_t[:, k, :],
                                 xt_t[:, k, :], start=(k == 0), stop=(k == 15))
            t0 = sb.tile([DR, TS], F32, tag="t0", bufs=1)
            t1 = sb.tile([DR, TS], F32, tag="t1", bufs=1)
            nc.vector.tensor_mul(t0, pe[0:DR, :], cos_t)
            nc.vector.tensor_mul(t1, pe[DR:128, :], sin_t)
            kp = sb.tile([DR, TS], F8, tag="kp", bufs=1)
            with nc.allow_low_precision(reason="fp8 roped k_pe"):
                nc.vector.tensor_add(kp, t0, t1)
            nc.scalar.dma_start(out=kpe[:, :], in_=kp)

            q_var = ps.tile([1, TS], F32, tag="row", bufs=1, name="q_var")
            for m in range(4):
                nc.tensor.matmul(q_var, ones_col, q_sqs[m],
                                 start=(m == 0), stop=(m == 3))
            q_inv = sb.tile([1, TS], F32R, tag="inv", bufs=2, name="q_inv")
            nc.scalar.activation(q_inv, q_var, AF.Abs_reciprocal_sqrt,
                                 scale=1.0 / 512.0, bias=eps_t[:, :])

            # kv latents: m=0 reuses the pe bank (already consumed) so it can
            # run while the q normalization chain drains; m=1..3 use the q acc
            # banks freed by the q muls
            kv_tags = ["mm4", "mm0", "mm1", "mm2"]
            kv_accs, kv_sqs = [], []
            q_done = [False]

            def q_norm_tail():
                q_bc = ps.tile([128, TS], F32, tag="bc", bufs=1, name="q_bc")
                nc.tensor.matmul(q_bc, ones_row, q_inv, start=True, stop=True)
                q_bcs = sb.tile([128, TS], BF16, tag="bcs", bufs=2, name="q_bcs")
                nc.scalar.copy(q_bcs, q_bc)
                o_all = sb.tile([128, 4, TS], BF16, tag="no", bufs=1)
                for m in range(4):
                    with nc.allow_low_precision(reason="bf16 latents"):
                        nc.vector.tensor_mul(o_all[:, m, :], q_accs[m], q_bcs)
                out_r = cqn[:, :].rearrange("(m p) t -> p m t", p=128)
                nc.scalar.dma_start(out=out_r, in_=o_all)

            kv_var = ps.tile([1, TS], F32, tag="row", bufs=1, name="kv_var")
            for m in range(4):
                acc = ps.tile([128, TS], F32, tag=kv_tags[m], bufs=1, name="acc")
                for k in range(16):
                    nc.tensor.matmul(acc, wkva_t[:, k, m * 128:(m + 1) * 128],
                                     xt_t[:, k, :], start=(k == 0), stop=(k == 15))
                    if m == 0 and k == 3 and not q_done[0]:
                        # q normalization PE op slides into the kv stream
                        q_norm_tail()
                        q_done[0] = True
                kv_accs.append(acc)
                sq = sb.tile([128, TS], BF16, tag=f"sq{m}", bufs=1, name="sq")
                nc.scalar.activation(sq, acc, AF.Square)
                kv_sqs.append(sq)
                if m >= 2:
                    # fold variance matmuls for earlier tiles into the stream
                    nc.tensor.matmul(kv_var, ones_col, kv_sqs[m - 2],
                                     start=(m == 2), stop=False)
            for m in (2, 3):
                nc.tensor.matmul(kv_var, ones_col, kv_sqs[m],
                                 start=False, stop=(m == 3))
            kv_inv = sb.tile([1, TS], F32R, tag="inv", bufs=2, name="kv_inv")
            nc.scalar.activation(kv_inv, kv_var, AF.Abs_reciprocal_sqrt,
                                 scale=1.0 / 512.0, bias=eps_t[:, :])
            kv_bc = ps.tile([128, TS], F32, tag="bc", bufs=1, name="kv_bc")
            nc.tensor.matmul(kv_bc, ones_row, kv_inv, start=True, stop=True)
            kv_bcs = sb.tile([128, TS], BF16, tag="bcs", bufs=2, name="kv_bcs")
            nc.scalar.copy(kv_bcs, kv_bc)
            o_kv = []
            for m in range(4):
                o = sb.tile([128, TS], BF16, tag=f"o{m}", bufs=1, name="o")
                with nc.allow_low_precision(reason="bf16 latents"):
                    nc.vector.tensor_mul(o, kv_accs[m], kv_bcs)
                o_kv.append(o)

            # kn^T and v projections for ALL heads on this core's tokens
            # (moves this PE + cast work off the attention launch, where the
            # scalar engines are the bottleneck)
            knT_r = knT[:, :].rearrange("(d p) t -> p d t", p=128)
            for dt in range(16):
                acc = ps.tile([128, TS], F32, tag=f"mm{dt % 4}", bufs=1,
                              name="acc")
                for m in range(4):
                    nc.tensor.matmul(acc, wkb_t[:, m, dt * 128:(dt + 1) * 128],
                                     o_kv[m], start=(m == 0), stop=(m == 3))
                kc = sb.tile([128, TS], F8, tag=f"knc{dt % 4}", bufs=2, name="kc")
                with nc.allow_low_precision(reason="fp8 k nope"):
                    (nc.vector.tensor_copy if dt % 2 else nc.scalar.copy)(kc, acc)
                nc.scalar.dma_start(out=knT_r[:, dt, :], in_=kc)
            vout_r = vout[:, :].rearrange("(t4 p) v -> p t4 v", p=128)
            for t4 in range(4):
                vc = sb.tile([128, 4, 512], BF16, tag="vc", bufs=2, name="vc")
                for vd in range(4):
                    acc = ps.tile([128, 512], F32, tag=f"mm{vd % 4}", bufs=1,
                                  name="acc", padded_shape=[128, TS])
                    for m in range(4):
                        nc.tensor.matmul(acc, o_kv[m][:, t4 * 128:(t4 + 1) * 128],
                                         wvb_t[:, m, vd * 512:(vd + 1) * 512],
                                         start=(m == 0), stop=(m == 3))
                    (nc.vector.tensor_copy if vd % 2 else nc.scalar.copy)(
                        vc[:, vd, :], acc)
                    nc.scalar.dma_start(out=vout_r[:, t4, vd * 512:(vd + 1) * 512],
                                        in_=vc[:, vd, :])
    nc.compile()
    return nc


# --------------------------------------------------------------------------
# Launch 2: B-projections + q RoPE + causal attention + wo (head-sharded)
# --------------------------------------------------------------------------
def build_k2():
    nc = bacc.Bacc("TRN2", target_bir_lowering=False)
    cqn = nc.dram_tensor("cqn", [RQ, T], BF16, kind="ExternalInput")
    knin = nc.dram_tensor("knin", [128, 2, T], F8, kind="ExternalInput")
    vin = nc.dram_tensor("vin", [T, 2 * DV], BF16, kind="ExternalInput")
    kpin = nc.dram_tensor("kpin", [128, 2, T], F8, kind="ExternalInput")
    zf8 = nc.dram_tensor("zf8", [DR, S], F8, kind="ExternalInput")
    # wqb m-tiles: [nope_h0][nope_h1][pe_h0|pe_h1][sw_h0|sw_h1]
    wqb = nc.dram_tensor("wqb", [128, 4, 512], BF16, kind="ExternalInput")
    wop = nc.dram_tensor("wop", [128, 2, DIM], BF16, kind="ExternalInput")
    # csf[:, 0, :] = [cosF; cosF], csf[:, 1, :] = [sinF; sinF]
    csf = nc.dram_tensor("csf", [128, 2, S], BF16, kind="ExternalInput")
    maskp = nc.dram_tensor("maskp", [128, 128], BF16, kind="ExternalInput")
    onec = nc.dram_tensor("onec", [128, 1], BF16, kind="ExternalInput")
    oner = nc.dram_tensor("oner", [1, 128], F32R, kind="ExternalInput")
    out = nc.dram_tensor("out", [T, DIM], BF16, kind="ExternalOutput")

    cqn_r = cqn[:, :].rearrange("(k p) t -> p k t", p=128)

    with tile.TileContext(nc) as tc:
        with tc.tile_pool(name="const", bufs=1) as cpool, \
             tc.tile_pool(name="perb", bufs=2) as perb, \
             tc.tile_pool(name="sb", bufs=2) as sb, \
             tc.tile_pool(name="ps", bufs=1, space="PSUM") as ps:
            ones_col = cpool.tile([128, 1], BF16)
            ones_row = cpool.tile([1, 128], F32R)
            wqb_t = cpool.tile([128, 4, 512], BF16)
            wop_t = cpool.tile([128, 2, DIM], BF16)
            cs_t = cpool.tile([128, 2, S], BF16)
            mask_t = cpool.tile([128, 128], BF16)

            consts_loaded = [False]
            # per-batch activation tiles, allocated lazily (bufs=2 overlaps
            # the two batches)
            btiles = {}
            # prefetched cq/ckv input tiles keyed (b, tt)
            pref = {}

            def get_bt(bb):
                # q8/k8: [latent 128, head, pair, token]; pair 0 = nope dims,
                # pair 1 = [rope(64); zeros] (h0) or [zeros; rope(64)] (h1)
                if bb not in btiles:
                    q8_t = perb.tile([128, 2, 2, S], F8, tag="qn", name="q8_t")
                    k8_t = perb.tile([128, 2, 2, S], F8, tag="kn", name="k8_t")
                    v_t = perb.tile([128, 16, 256], BF16, tag="v", name="v_t")
                    o_t = perb.tile([128, 2, S], BF16, tag="o", name="o_t")
                    btiles[bb] = (q8_t, k8_t, v_t, o_t)
                return btiles[bb]

            def load_tt(bb, tt):
                g0 = bb * S + tt * 512
                cq_t = sb.tile([128, 4, 512], BF16, tag="cq", bufs=3, name="cq_t")
                if not consts_loaded[0]:
                    # split first loads so the first projection matmuls can
                    # start as soon as the k=0..1 halves land
                    nc.sync.dma_start(out=wqb_t[:, 0:2, :], in_=wqb[:, 0:2, :])
                    nc.sync.dma_start(out=cq_t[:, 0:2, :],
                                      in_=cqn_r[:, 0:2, g0:g0 + 512])
                    nc.sync.dma_start(out=wqb_t[:, 2:4, :], in_=wqb[:, 2:4, :])
                    nc.sync.dma_start(out=cq_t[:, 2:4, :],
                                      in_=cqn_r[:, 2:4, g0:g0 + 512])
                else:
                    nc.sync.dma_start(out=cq_t, in_=cqn_r[:, :, g0:g0 + 512])
                if bb == 0:
                    ssl = slice(tt * 512, (tt + 1) * 512)
                    nc.sync.dma_start(out=cs_t[:, :, ssl], in_=csf[:, :, ssl])
                if not consts_loaded[0]:
                    nc.sync.dma_start(out=mask_t, in_=maskp[:, :])
                    nc.sync.dma_start(out=ones_col, in_=onec[:, :])
                    nc.sync.dma_start(out=ones_row, in_=oner[:, :])
                    consts_loaded[0] = True
                if tt == 0:
                    q8_t, k8_t, v_t = get_bt(bb)[:3]
                    bsl = slice(bb * S, (bb + 1) * S)
                    nc.sync.dma_start(out=q8_t[DR:128, 0, 1, :], in_=zf8[:, :])
                    nc.sync.dma_start(out=q8_t[0:DR, 1, 1, :], in_=zf8[:, :])
                    nc.sync.dma_start(out=k8_t[:, :, 0, :], in_=knin[:, :, bsl])
                    nc.sync.dma_start(out=k8_t[:, :, 1, :], in_=kpin[:, :, bsl])
                    nc.sync.dma_start(
                        out=v_t,
                        in_=vin[bsl, :].rearrange("(kt p) v -> p kt v", p=128))
                pref[(bb, tt)] = cq_t

            wo_queue = []
            outs_map = {}

            def wo_chunk(bb, o_tt, t16, ch):
                tsl = slice(t16 * 128, (t16 + 1) * 128)
                acc = ps.tile([128, 512], F32, tag="mm", bufs=4, name="acc")
                for hh in range(2):
                    nc.tensor.matmul(acc, o_tt[:, hh, tsl],
                                     wop_t[:, hh, ch * 512:(ch + 1) * 512],
                                     start=(hh == 0), stop=(hh == 1))
                if ch == 0:
                    outs_map[(bb, t16)] = sb.tile([128, DIM], BF16, tag="outs",
                                                  bufs=2, name="outs")
                outs = outs_map[(bb, t16)]
                eng = nc.scalar.copy if ch == 0 else nc.vector.tensor_copy
                eng(outs[:, ch * 512:(ch + 1) * 512], acc)
                if ch % 2 == 1:
                    half = slice((ch - 1) * 512, (ch + 1) * 512)
                    nc.sync.dma_start(
                        out=out[bb * S + t16 * 128:bb * S + (t16 + 1) * 128, half],
                        in_=outs[:, half])
                    if ch == 3:
                        del outs_map[(bb, t16)]

            for b in range(B):
                q8_t, k8_t, v_t, o_t = get_bt(b)

                # ---- B projection of one 512-token tile ----
                def proj_tt(tt):
                    sl = slice(tt * 512, (tt + 1) * 512)
                    cq_t = pref.pop((b, tt))
                    if tt == 1 and b == 0:
                        nc.sync.dma_start(out=wop_t, in_=wop[:, :, :])

                    accs = []
                    for m in range(4):  # h0 nope, h1 nope, pe pair, swap pair
                        acc = ps.tile([128, 512], F32, tag="mm", bufs=4,
                                      name="acc")
                        for k in range(4):
                            nc.tensor.matmul(acc, wqb_t[:, k, m * 128:(m + 1) * 128],
                                             cq_t[:, k, :], start=(k == 0), stop=(k == 3))
                        accs.append(acc)
                        if m < 2:
                            with nc.allow_low_precision(reason="fp8 q nope"):
                                nc.vector.tensor_copy(q8_t[:, m, 0, sl], acc)
                    # RoPE for both heads in one sweep
                    t0 = sb.tile([128, 512], BF16, tag="t0", bufs=2)
                    t1 = sb.tile([128, 512], BF16, tag="t1", bufs=2)
                    with nc.allow_low_precision(reason="fp8 roped q_pe"):
                        nc.vector.tensor_mul(t0, accs[2], cs_t[:, 0, sl])
                        nc.vector.tensor_mul(t1, accs[3], cs_t[:, 1, sl])
                        nc.vector.tensor_add(q8_t[0:DR, 0, 1, sl],
                                             t0[0:DR, :], t1[0:DR, :])
                        nc.vector.tensor_add(q8_t[DR:128, 1, 1, sl],
                                             t0[DR:128, :], t1[DR:128, :])

                # ---- causal attention (scores transposed: [k, q]) ----
                def normalize1(pend):
                    hh, lacc_p, oacc_p, qsl_p = pend
                    inv = sb.tile([1, 512], F32R, tag="inv", bufs=2)
                    with nc.allow_low_precision(reason="fp32r rounding of softmax denom"):
                        nc.vector.reciprocal(inv, lacc_p)
                    return (hh, inv, oacc_p, qsl_p)

                def normalize2(pend):
                    hh, inv, oacc_p, qsl_p = pend
                    bc = ps.tile([128, 512], F32, tag="mm", bufs=4)
                    nc.tensor.matmul(bc, ones_row, inv, start=True, stop=True)
                    bcs = sb.tile([128, 512], BF16, tag="bcs", bufs=2)
                    nc.scalar.copy(bcs, bc)
                    with nc.allow_low_precision(reason="bf16 attn output"):
                        nc.vector.tensor_mul(o_t[:, hh, qsl_p], oacc_p, bcs)

                pend_box = [None]

                def attn_qt(qt):
                    for h in range(2):
                        qsl = slice(qt * 512, (qt + 1) * 512)
                        nkt = 4 * qt + 4
                        lacc = ps.tile([1, 512], F32, tag="row", bufs=2, name="lacc")
                        oacc = ps.tile([128, 512], F32, tag="pv", bufs=2, name="oacc")

                        lst = {"started": False, "pend": None}

                        def lacc_mm(src, off_p, w_p, last):
                            nc.tensor.matmul(lacc[:, off_p:512], ones_col,
                                             src[:, :w_p],
                                             start=(not lst["started"]), stop=last)
                            lst["started"] = True

                        def consume(prev_e):
                            et_p, off_p, w_p, kt_p = prev_e
                            last = (kt_p == nkt - 1)
                            j = kt_p - 4 * qt
                            if 0 <= j < 4:
                                # only the 128-col diagonal block needs masking
                                with nc.allow_low_precision(reason="bf16 probs"):
                                    nc.vector.tensor_mul(et_p[:, 0:128],
                                                         et_p[:, 0:128], mask_t)
                            if off_p == 0 and not last:
                                # pair full-width tiles: one denominator
                                # matmul per two tiles (shallow DVE add)
                                if lst["pend"] is None:
                                    lst["pend"] = et_p
                                else:
                                    es = sb.tile([128, 512], BF16, tag="es",
                                                 bufs=3)
                                    with nc.allow_low_precision(reason="bf16 denom"):
                                        nc.vector.tensor_add(es, lst["pend"], et_p)
                                    lst["pend"] = None
                                    lacc_mm(es, 0, 512, False)
                            else:
                                if lst["pend"] is not None:
                                    lacc_mm(lst["pend"], 0, 512, False)
                                    lst["pend"] = None
                                lacc_mm(et_p, off_p, w_p, last)
                            nc.tensor.matmul(oacc[:, off_p:512],
                                             v_t[:, kt_p, h * 128:(h + 1) * 128],
                                             et_p[:, :w_p],
                                             start=(kt_p == 0), stop=(kt_p == nkt - 1))

                        pend2 = []
                        for kt in range(nkt):
                            ksl = slice(kt * 128, (kt + 1) * 128)
                            j = kt - 4 * qt
                            # columns of this q-tile that can be unmasked:
                            off = 0 if j < 1 else 128 * j
                            w = 512 - off
                            qs2 = slice(qt * 512 + off, (qt + 1) * 512)
                            sc = ps.tile([128, 512], F32, tag="mm", bufs=4, name="sc")
                            nc.tensor.matmul(sc[:, :w], k8_t[:, h, :, ksl],
                                             q8_t[:, h, :, qs2],
                                             start=True, stop=True,
                                             perf_mode=mybir.MatmulPerfMode.DoubleRow)
                            if len(pend2) >= 3:
                                consume(pend2.pop(0))
                            et = sb.tile([128, 512], BF16, tag="exp", bufs=6)
                            nc.scalar.activation(et[:, :w], sc[:, :w], AF.Exp,
                                                 scale=SCALE)
                            pend2.append((et, off, w, kt))
                            if kt == 0:
                                if pend_box[0] is not None:
                                    pend_box[0] = normalize1(pend_box[0])
                            elif kt == 2 and pend_box[0] is not None:
                                normalize2(pend_box[0])
                                pend_box[0] = None
                            elif kt >= 3 and wo_queue:
                                wo_chunk(*wo_queue.pop(0))
                                if len(wo_queue) > 16:
                                    wo_chunk(*wo_queue.pop(0))
                        for e in pend2:
                            consume(e)
                        pend_box[0] = (h, lacc, oacc, qsl)
                        if h == 1:
                            wo_queue.extend((b, o_t, t16, ch) for t16 in
                                            range(qt * 4, qt * 4 + 4)
                                            for ch in range(4))

                # software pipeline: loads lead projections, projections lead
                # attention by one tile; next batch prefetched during attention
                if b == 0:
                    load_tt(0, 0)
                    load_tt(0, 1)
                proj_tt(0)
                load_tt(b, 2)
                proj_tt(1)
                load_tt(b, 3)
                attn_qt(0)
                proj_tt(2)
                attn_qt(1)
                proj_tt(3)
                if b == 0:
                    load_tt(1, 0)
                    load_tt(1, 1)
                attn_qt(2)
                attn_qt(3)
                if pend_box[0] is not None:
                    normalize2(normalize1(pend_box[0]))
                    pend_box[0] = None
                if b == B - 1:
                    for e in wo_queue:
                        wo_chunk(*e)
                    wo_queue.clear()

    nc.compile()
    return nc


# --------------------------------------------------------------------------
# Host-side data prep
# --------------------------------------------------------------------------
def _pack(wT, ktiles):
    """(ktiles*128, M) -> (128, ktiles, M) with [p, k, m] = wT[k*128+p, m]."""
    K, M = wT.shape
    assert K == ktiles * 128
    return np.ascontiguousarray(
        wT.reshape(ktiles, 128, M).transpose(1, 0, 2)).astype(NPBF)


def _swap_pairs(a, axis):
    idx = np.arange(a.shape[axis])
    idx = idx.reshape(-1, 2)[:, ::-1].reshape(-1)
    return np.take(a, idx, axis=axis)


def _prep(inputs):
    x = np.asarray(inputs["x"], dtype=np.float32)
    f = np.asarray(inputs["freqs_cis"], dtype=np.float32)
    wq_a = np.asarray(inputs["wq_a"], dtype=np.float32)
    wq_b = np.asarray(inputs["wq_b"], dtype=np.float32)
    q_norm_w = np.asarray(inputs["q_norm_w"], dtype=np.float32)
    wkv_a = np.asarray(inputs["wkv_a"], dtype=np.float32)
    kv_norm_w = np.asarray(inputs["kv_norm_w"], dtype=np.float32)
    wkv_b = np.asarray(inputs["wkv_b"], dtype=np.float32)
    wo = np.asarray(inputs["wo"], dtype=np.float32)

    xT = np.ascontiguousarray(x.reshape(T, DIM).T).astype(NPBF)  # (DIM, T)

    cos = f[:, :, 0].T  # (32, S)
    sin = f[:, :, 1].T
    cosF = np.empty((DR, S), np.float32)
    sinF = np.empty((DR, S), np.float32)
    cosF[0::2] = cos
    cosF[1::2] = cos
    sinF[0::2] = -sin
    sinF[1::2] = sin

    wqaT = wq_a.T                       # (DIM, RQ)
    wkvaT = wkv_a.T                     # (DIM, RKV+DR)
    pe = wkvaT[:, RKV:RKV + DR]
    wqa_p = _pack(wqaT, 16)
    wkva_p = _pack(wkvaT[:, :RKV], 16)
    wkvap_p = _pack(np.concatenate([pe, _swap_pairs(pe, 1)], axis=1), 16)
    one_c = np.ones((128, 1), NPBF)

    wqbT = (wq_b * q_norm_w[None, :]).T       # (RQ, H*DQK)
    wkvbT = (wkv_b * kv_norm_w[None, :]).T    # (RKV, H*(DN+DV))
    woT = wo.T                                # (H*DV, DIM)
    kn_cols = np.concatenate(
        [wkvbT[:, h * (DN + DV):h * (DN + DV) + DN] for h in range(H)], axis=1)
    v_cols = np.concatenate(
        [wkvbT[:, h * (DN + DV) + DN:(h + 1) * (DN + DV)] for h in range(H)],
        axis=1)
    wkbf_p = _pack(kn_cols, 4)
    wvbf_p = _pack(v_cols, 4)

    k1_maps = []
    for c in range(NCORES):
        t0 = c * TS
        srange = slice(t0 % S, t0 % S + TS)
        k1_maps.append({
            "xt": np.ascontiguousarray(xT[:, t0:t0 + TS]),
            "wqa": wqa_p, "wkva": wkva_p, "wkvap": wkvap_p,
            "wkbf": wkbf_p, "wvbf": wvbf_p,
            "cosk": np.ascontiguousarray(cosF[:, srange]),
            "sink": np.ascontiguousarray(sinF[:, srange]),
            "onec": one_c,
            "oner": np.ones((1, 128), np.float32),
        })

    # mask block: [p, c] = 1 if c >= p (valid within the 128-col diag block)
    mask = (np.arange(128)[None, :] >= np.arange(128)[:, None]).astype(NPBF)

    csf = np.empty((128, 2, S), np.float32)
    csf[0:DR, 0] = cosF
    csf[DR:128, 0] = cosF
    csf[0:DR, 1] = sinF
    csf[DR:128, 1] = sinF
    csf = csf.astype(NPBF)

    k2_maps = []
    for c in range(NCORES):
        h0, h1 = 2 * c, 2 * c + 1
        nope, pe_q, sw_q = [], [], []
        for hh in (h0, h1):
            base = hh * DQK
            nope.append(wqbT[:, base:base + DN])
            p = wqbT[:, base + DN:base + DQK]
            pe_q.append(p)
            sw_q.append(_swap_pairs(p, 1))
        # m-tiles: [nope_h0][nope_h1][pe_h0|pe_h1][sw_h0|sw_h1]
        q_ext = np.concatenate(nope + pe_q + sw_q, axis=1)    # (512, 512)
        worows = np.concatenate([woT[hh * DV:(hh + 1) * DV] for hh in (h0, h1)], axis=0)
        k2_maps.append({
            "wqb": _pack(q_ext, 4),
            "wop": _pack(worows, 2),
            "csf": csf, "maskp": mask,
            "onec": one_c,
            "oner": np.ones((1, 128), np.float32),
            "zf8": np.zeros((DR, S), NPF8),
        })
    return k1_maps, k2_maps


def _get(name, builder):
    if name not in _CACHE:
        _CACHE[name] = builder()
    return _CACHE[name]


def _run(inputs, trace=False):
    k1_maps, k2_maps = _prep(inputs)
    nc1 = _get("k1", build_k1)
    r1 = run_bass_kernel_spmd(nc1, k1_maps, core_ids=list(range(NCORES)), trace=trace)

    cqn = np.concatenate([r1.results[c]["cqn"] for c in range(NCORES)], axis=1)
    knT = np.concatenate([r1.results[c]["knT"] for c in range(NCORES)], axis=1)
    vfull = np.concatenate([r1.results[c]["vout"] for c in range(NCORES)], axis=0)
    kpe = np.concatenate([r1.results[c]["kpe"] for c in range(NCORES)], axis=1)
    # pair-1 of the k8 tiles, prebuilt: [kp; 0] for h0, [0; kp] for h1
    kpin = np.zeros((128, 2, T), NPF8)
    kpin[0:DR, 0] = kpe
    kpin[DR:128, 1] = kpe
    for c, m in enumerate(k2_maps):
        m["cqn"] = cqn
        m["kpin"] = kpin
        m["knin"] = np.ascontiguousarray(
            knT[c * 256:(c + 1) * 256].reshape(2, 128, T).transpose(1, 0, 2))
        m["vin"] = np.ascontiguousarray(vfull[:, c * 256:(c + 1) * 256])

    nc2 = _get("k2", build_k2)
    r2 = run_bass_kernel_spmd(nc2, k2_maps, core_ids=list(range(NCORES)), trace=trace)

    acc = r2.results[0]["out"].astype(np.float32)
    for c in range(1, NCORES):
        acc = acc + r2.results[c]["out"].astype(np.float32)
    return acc.reshape(B, S, DIM), (r1, r2)


def kernel(**inputs) -> np.ndarray:
    out, _ = _run(inputs)
    return out


# revision 54
# speedup vs baseline: 1.0420x; 1.0420x over previous
"""MLA-style attention (nn_Attention_15496242004691) on 8 trn2 NeuronCores.

Strategy (v3):
  Launch 1 (token-sharded, 512 tokens/core): A projections (bf16 matmuls,
    fp32 PSUM), RMSNorm (norm weights folded into the B projections on
    host), RoPE of k_pe (pair-swap folded into an extended wkv_a on host),
    plus the token-shardable halves of the B projections: k_nope^T (fp8)
    and v (bf16) for ALL heads. Emits cqn (bf16), knT/kpe (fp8), v (bf16).
  Launch 2 (head-sharded, 2 heads/core): q B-projection + q RoPE into a
    paired fp8 layout, causal attention with transposed scores
    ([k, q] tiles), output projection; host sums 8 bf16 partials in fp32.

Score matmuls run as fp8e4m3 DoubleRow (contraction pairs
[nope(128); rope(64)+zeros], 0.5 PE cycles/row); everything else is bf16
(1 cycle/row, half the DMA/SBUF traffic of fp32r). Softmax denominators:
exp tiles pair-summed on DVE, then ones-column matmuls into a PSUM row.
Elementwise work is balanced across ACT (exp + casts) and DVE (masks,
denominator pairs, normalize, casts); attention is software-pipelined 3
score tiles deep with wo chunks interleaved to fill PE bubbles. DMAs are
batched into few large descriptors (HWDGE serializes per instruction).
"""

import numpy as np
import ml_dtypes

import concourse.bass as bass
import concourse.mybir as mybir
import concourse.tile as tile
from concourse import bacc
from concourse.bass_utils import run_bass_kernel_spmd

F32 = mybir.dt.float32
F32R = mybir.dt.float32r
BF16 = mybir.dt.bfloat16
F8 = mybir.dt.float8e4
AF = mybir.ActivationFunctionType

NPBF = ml_dtypes.bfloat16
NPF8 = ml_dtypes.float8_e4m3

B, S, DIM, H = 2, 2048, 2048, 16
NCORES = 8
HPC = H // NCORES  # heads per core = 2
RQ = RKV = 512
DN, DR, DV, DQK = 128, 64, 128, 192
EPS = 1e-6
SCALE = DQK ** -0.5
T = B * S          # 4096 tokens
TS = T // NCORES   # 512 tokens per core in launch 1

_CACHE = {}


# --------------------------------------------------------------------------
# Launch 1: A-projections + RMSNorm + k_pe RoPE (token-sharded)
# --------------------------------------------------------------------------
def build_k1():
    nc = bacc.Bacc("TRN2", target_bir_lowering=False)
    xt = nc.dram_tensor("xt", [DIM, TS], BF16, kind="ExternalInput")
    wqa = nc.dram_tensor("wqa", [128, 16, RQ], BF16, kind="ExternalInput")
    wkva = nc.dram_tensor("wkva", [128, 16, RKV], BF16, kind="ExternalInput")
    wkvap = nc.dram_tensor("wkvap", [128, 16, 2 * DR], BF16, kind="ExternalInput")
    cosk = nc.dram_tensor("cosk", [DR, TS], F32, kind="ExternalInput")
    sink = nc.dram_tensor("sink", [DR, TS], F32, kind="ExternalInput")
    onec = nc.dram_tensor("onec", [128, 1], BF16, kind="ExternalInput")
    oner = nc.dram_tensor("oner", [1, 128], F32R, kind="ExternalInput")
    wkbf = nc.dram_tensor("wkbf", [128, 4, H * DN], BF16, kind="ExternalInput")
    wvbf = nc.dram_tensor("wvbf", [128, 4, H * DV], BF16, kind="ExternalInput")
    wqbe = nc.dram_tensor("wqbe", [128, 4, 4096], BF16, kind="ExternalInput")
    cosd = nc.dram_tensor("cosd", [128, TS], BF16, kind="ExternalInput")
    sind = nc.dram_tensor("sind", [128, TS], BF16, kind="ExternalInput")
    q8n = nc.dram_tensor("q8n", [128, H, TS], F8, kind="ExternalOutput")
    q8r = nc.dram_tensor("q8r", [128, H // 2, TS], F8, kind="ExternalOutput")
    knT = nc.dram_tensor("knT", [H * DN, TS], F8, kind="ExternalOutput")
    vout = nc.dram_tensor("vout", [TS, H * DV], BF16, kind="ExternalOutput")
    kpe = nc.dram_tensor("kpe", [DR, TS], F8, kind="ExternalOutput")

    with tile.TileContext(nc) as tc:
        with tc.tile_pool(name="const", bufs=1) as cpool, \
             tc.tile_pool(name="sb", bufs=2) as sb, \
             tc.tile_pool(name="ps", bufs=1, space="PSUM") as ps:
            ones_col = cpool.tile([128, 1], BF16)
            ones_row = cpool.tile([1, 128], F32R)
            eps_t = cpool.tile([1, 1], F32)
            nc.vector.memset(eps_t, EPS)

            xt_t = cpool.tile([128, 16, TS], BF16)
            xt_r = xt[:, :].rearrange("(k p) t -> p k t", p=128)
            wqa_t = cpool.tile([128, 16, RQ], BF16)
            wkva_t = cpool.tile([128, 16, RKV], BF16)
            wkvap_t = cpool.tile([128, 16, 2 * DR], BF16)
            cos_t = cpool.tile([DR, TS], F32)
            sin_t = cpool.tile([DR, TS], F32)
            # few big DMAs (HWDGE serializes per-instruction); graduated
            # chunks so the first matmuls can start early
            chunks = [(0, 1), (1, 2), (2, 4), (4, 8), (8, 16)]
            for lo, hi in chunks:
                ksl = slice(lo, hi)
                nc.sync.dma_start(out=wqa_t[:, ksl, :], in_=wqa[:, ksl, :])
                nc.sync.dma_start(out=xt_t[:, ksl, :], in_=xt_r[:, ksl, :])
                if hi == 4:
                    # pe columns early: the kv pe tile runs right after q
                    nc.sync.dma_start(out=wkvap_t, in_=wkvap[:, :, :])
            for c2 in range(2):
                ksl = slice(8 * c2, 8 * c2 + 8)
                nc.sync.dma_start(out=wkva_t[:, ksl, :], in_=wkva[:, ksl, :])
            nc.sync.dma_start(out=cos_t, in_=cosk[:, :])
            nc.sync.dma_start(out=sin_t, in_=sink[:, :])
            nc.sync.dma_start(out=ones_col, in_=onec[:, :])
            nc.sync.dma_start(out=ones_row, in_=oner[:, :])
            wkb_t = cpool.tile([128, 4, H * DN], BF16)
            wvb_t = cpool.tile([128, 4, H * DV], BF16)
            for c2 in range(2):
                csl = slice(H * DN // 2 * c2, H * DN // 2 * (c2 + 1))
                nc.sync.dma_start(out=wkb_t[:, :, csl], in_=wkbf[:, :, csl])
                nc.sync.dma_start(out=wvb_t[:, :, csl], in_=wvbf[:, :, csl])
            wqbe_t = cpool.tile([128, 4, 4096], BF16)
            cosd_t = cpool.tile([128, TS], BF16)
            sind_t = cpool.tile([128, TS], BF16)
            nc.sync.dma_start(out=cosd_t, in_=cosd[:, :])
            nc.sync.dma_start(out=sind_t, in_=sind[:, :])
            for c4 in range(4):
                csl = slice(1024 * c4, 1024 * (c4 + 1))
                nc.sync.dma_start(out=wqbe_t[:, :, csl], in_=wqbe[:, :, csl])

            # q path: k-outer (matches the DMA chunk arrival order)
            q_accs = []
            for m in range(4):
                acc = ps.tile([128, TS], F32, tag=f"mm{m}", bufs=1, name="acc")
                q_accs.append(acc)
            for k in range(16):
                for m in range(4):
                    nc.tensor.matmul(q_accs[m], wqa_t[:, k, m * 128:(m + 1) * 128],
                                     xt_t[:, k, :], start=(k == 0), stop=(k == 15))
            q_sqs = []
            for m in range(4):
                sq = sb.tile([128, TS], BF16, tag=f"sq{m}", bufs=1, name="sq")
                nc.scalar.activation(sq, q_accs[m], AF.Square)
                q_sqs.append(sq)

            # kv pe tile: its matmuls hide the q Square/rsqrt chain
            pe = ps.tile([128, TS], F32, tag="mm4", bufs=1, name="pe")
            for k in range(16):
                nc.tensor.matmul(pe, wkvap_t[:, k, :],
                                 xt_t[:, k, :], start=(k == 0), stop=(k == 15))
            t0 = sb.tile([DR, TS], F32, tag="t0", bufs=1)
            t1 = sb.tile([DR, TS], F32, tag="t1", bufs=1)
            nc.vector.tensor_mul(t0, pe[0:DR, :], cos_t)
            nc.vector.tensor_mul(t1, pe[DR:128, :], sin_t)
            kp = sb.tile([DR, TS], F8, tag="kp", bufs=1)
            with nc.allow_low_precision(reason="fp8 roped k_pe"):
                nc.vector.tensor_add(kp, t0, t1)
            nc.scalar.dma_start(out=kpe[:, :], in_=kp)

            q_var = ps.tile([1, TS], F32, tag="row", bufs=1, name="q_var")
            for m in range(4):
                nc.tensor.matmul(q_var, ones_col, q_sqs[m],
                                 start=(m == 0), stop=(m == 3))
            q_inv = sb.tile([1, TS], F32R, tag="inv", bufs=2, name="q_inv")
            nc.scalar.activation(q_inv, q_var, AF.Abs_reciprocal_sqrt,
                                 scale=1.0 / 512.0, bias=eps_t[:, :])

            # kv latents: m=0 reuses the pe bank (already consumed) so it can
            # run while the q normalization chain drains; m=1..3 use the q acc
            # banks freed by the q muls
            kv_tags = ["mm4", "mm0", "mm1", "mm2"]
            kv_accs, kv_sqs = [], []
            q_done = [False]
            o_box = [None]

            def q_norm_tail():
                q_bc = ps.tile([128, TS], F32, tag="bc", bufs=1, name="q_bc")
                nc.tensor.matmul(q_bc, ones_row, q_inv, start=True, stop=True)
                q_bcs = sb.tile([128, TS], BF16, tag="bcs", bufs=2, name="q_bcs")
                nc.scalar.copy(q_bcs, q_bc)
                o_all = sb.tile([128, 4, TS], BF16, tag="no", bufs=1)
                for m in range(4):
                    with nc.allow_low_precision(reason="bf16 latents"):
                        nc.vector.tensor_mul(o_all[:, m, :], q_accs[m], q_bcs)
                o_box[0] = o_all

            kv_var = ps.tile([1, TS], F32, tag="row", bufs=1, name="kv_var")
            for m in range(4):
                acc = ps.tile([128, TS], F32, tag=kv_tags[m], bufs=1, name="acc")
                for k in range(16):
                    nc.tensor.matmul(acc, wkva_t[:, k, m * 128:(m + 1) * 128],
                                     xt_t[:, k, :], start=(k == 0), stop=(k == 15))
                    if m == 0 and k == 3 and not q_done[0]:
                        # q normalization PE op slides into the kv stream
                        q_norm_tail()
                        q_done[0] = True
                kv_accs.append(acc)
                sq = sb.tile([128, TS], BF16, tag=f"sq{m}", bufs=1, name="sq")
                nc.scalar.activation(sq, acc, AF.Square)
                kv_sqs.append(sq)
                if m >= 2:
                    # fold variance matmuls for earlier tiles into the stream
                    nc.tensor.matmul(kv_var, ones_col, kv_sqs[m - 2],
                                     start=(m == 2), stop=False)
            for m in (2, 3):
                nc.tensor.matmul(kv_var, ones_col, kv_sqs[m],
                                 start=False, stop=(m == 3))
            kv_inv = sb.tile([1, TS], F32R, tag="inv", bufs=2, name="kv_inv")
            nc.scalar.activation(kv_inv, kv_var, AF.Abs_reciprocal_sqrt,
                                 scale=1.0 / 512.0, bias=eps_t[:, :])
            kv_bc = ps.tile([128, TS], F32, tag="bc", bufs=1, name="kv_bc")
            nc.tensor.matmul(kv_bc, ones_row, kv_inv, start=True, stop=True)
            kv_bcs = sb.tile([128, TS], BF16, tag="bcs", bufs=2, name="kv_bcs")
            nc.scalar.copy(kv_bcs, kv_bc)
            o_kv = []
            for m in range(4):
                o = sb.tile([128, TS], BF16, tag=f"o{m}", bufs=1, name="o")
                with nc.allow_low_precision(reason="bf16 latents"):
                    nc.vector.tensor_mul(o, kv_accs[m], kv_bcs)
                o_kv.append(o)

            # q B-projection (+RoPE) for ALL heads on this core's tokens;
            # wqbe cols: [nope_h x16][pe-pair x8][sw-pair x8]
            qtags = ["mm3", "mm4", "mm0", "mm1", "mm2"]
            for i in range(16):
                acc = ps.tile([128, TS], F32, tag=qtags[i % 5], bufs=1,
                              name="acc")
                for m in range(4):
                    nc.tensor.matmul(acc, wqbe_t[:, m, i * 128:(i + 1) * 128],
                                     o_box[0][:, m, :], start=(m == 0), stop=(m == 3))
                qc = sb.tile([128, TS], F8, tag=f"qc{i % 4}", bufs=2, name="qc")
                with nc.allow_low_precision(reason="fp8 q nope"):
                    (nc.vector.tensor_copy if i % 2 else nc.scalar.copy)(qc, acc)
                nc.scalar.dma_start(out=q8n[:, i, :], in_=qc)
            for i in range(8):
                acc_pe = ps.tile([128, TS], F32, tag=qtags[(2 * i) % 5], bufs=1,
                                 name="acc_pe")
                for m in range(4):
                    nc.tensor.matmul(acc_pe,
                                     wqbe_t[:, m, 2048 + i * 128:2048 + (i + 1) * 128],
                                     o_box[0][:, m, :], start=(m == 0), stop=(m == 3))
                acc_sw = ps.tile([128, TS], F32, tag=qtags[(2 * i + 1) % 5], bufs=1,
                                 name="acc_sw")
                for m in range(4):
                    nc.tensor.matmul(acc_sw,
                                     wqbe_t[:, m, 3072 + i * 128:3072 + (i + 1) * 128],
                                     o_box[0][:, m, :], start=(m == 0), stop=(m == 3))
                rt0 = sb.tile([128, TS], BF16, tag="rt0", bufs=2, name="rt0")
                rt1 = sb.tile([128, TS], BF16, tag="rt1", bufs=2, name="rt1")
                rq = sb.tile([128, TS], F8, tag=f"rq{i % 2}", bufs=2, name="rq")
                with nc.allow_low_precision(reason="fp8 roped q_pe"):
                    nc.vector.tensor_mul(rt0, acc_pe, cosd_t)
                    nc.vector.tensor_mul(rt1, acc_sw, sind_t)
                    nc.vector.tensor_add(rq, rt0, rt1)
                nc.scalar.dma_start(out=q8r[:, i, :], in_=rq)

            # kn^T and v projections for ALL heads on this core's tokens
            # (moves this PE + cast work off the attention launch, where the
            # scalar engines are the bottleneck)
            knT_r = knT[:, :].rearrange("(d p) t -> p d t", p=128)
            for dt in range(16):
                acc = ps.tile([128, TS], F32, tag=f"mm{dt % 4}", bufs=1,
                              name="acc")
                for m in range(4):
                    nc.tensor.matmul(acc, wkb_t[:, m, dt * 128:(dt + 1) * 128],
                                     o_kv[m], start=(m == 0), stop=(m == 3))
                kc = sb.tile([128, TS], F8, tag=f"knc{dt % 4}", bufs=2, name="kc")
                with nc.allow_low_precision(reason="fp8 k nope"):
                    (nc.vector.tensor_copy if dt % 2 else nc.scalar.copy)(kc, acc)
                nc.scalar.dma_start(out=knT_r[:, dt, :], in_=kc)
            vout_r = vout[:, :].rearrange("(t4 p) v -> p t4 v", p=128)
            for t4 in range(4):
                vc = sb.tile([128, 4, 512], BF16, tag="vc", bufs=2, name="vc")
                for vd in range(4):
                    acc = ps.tile([128, 512], F32, tag=f"mm{vd % 4}", bufs=1,
                                  name="acc", padded_shape=[128, TS])
                    for m in range(4):
                        nc.tensor.matmul(acc, o_kv[m][:, t4 * 128:(t4 + 1) * 128],
                                         wvb_t[:, m, vd * 512:(vd + 1) * 512],
                                         start=(m == 0), stop=(m == 3))
                    (nc.vector.tensor_copy if vd % 2 else nc.scalar.copy)(
                        vc[:, vd, :], acc)
                    nc.scalar.dma_start(out=vout_r[:, t4, vd * 512:(vd + 1) * 512],
                                        in_=vc[:, vd, :])
    nc.compile()
    return nc


# --------------------------------------------------------------------------
# Launch 2: B-projections + q RoPE + causal attention + wo (head-sharded)
# --------------------------------------------------------------------------
def build_k2():
    nc = bacc.Bacc("TRN2", target_bir_lowering=False)
    q8ni = nc.dram_tensor("q8ni", [128, 2, T], F8, kind="ExternalInput")
    q8ri = nc.dram_tensor("q8ri", [128, T], F8, kind="ExternalInput")
    knin = nc.dram_tensor("knin", [128, 2, T], F8, kind="ExternalInput")
    vin = nc.dram_tensor("vin", [T, 2 * DV], BF16, kind="ExternalInput")
    kpin = nc.dram_tensor("kpin", [128, 2, T], F8, kind="ExternalInput")
    wop = nc.dram_tensor("wop", [128, 2, DIM], BF16, kind="ExternalInput")
    maskp = nc.dram_tensor("maskp", [128, 128], BF16, kind="ExternalInput")
    onec = nc.dram_tensor("onec", [128, 1], BF16, kind="ExternalInput")
    oner = nc.dram_tensor("oner", [1, 128], F32R, kind="ExternalInput")
    out = nc.dram_tensor("out", [T, DIM], BF16, kind="ExternalOutput")

    with tile.TileContext(nc) as tc:
        with tc.tile_pool(name="const", bufs=1) as cpool, \
             tc.tile_pool(name="perb", bufs=2) as perb, \
             tc.tile_pool(name="sb", bufs=2) as sb, \
             tc.tile_pool(name="ps", bufs=1, space="PSUM") as ps:
            ones_col = cpool.tile([128, 1], BF16)
            ones_row = cpool.tile([1, 128], F32R)
            wop_t = cpool.tile([128, 2, DIM], BF16)
            mask_t = cpool.tile([128, 128], BF16)

            btiles = {}

            def load_b(bb):
                # q8/k8: [contraction 128, head, pair, token]; pair 0 = nope,
                # pair 1 = roped pe. The k side zero-pads the unused half of
                # each head's rope rows, so the q rope pair (holding BOTH
                # heads' rope values) is shared between the two heads.
                q8_t = perb.tile([128, 2, 2, S], F8, tag="qn", name="q8_t")
                k8_t = perb.tile([128, 2, 2, S], F8, tag="kn", name="k8_t")
                v_t = perb.tile([128, 16, 256], BF16, tag="v", name="v_t")
                o_t = perb.tile([128, 2, S], BF16, tag="o", name="o_t")
                btiles[bb] = (q8_t, k8_t, v_t, o_t)
                # batch 0 streams in four sequence chunks so the first score
                # matmuls only wait for the first quarter
                bsl = slice(bb * S, (bb + 1) * S)
                for hh in range(2):
                    nc.sync.dma_start(out=q8_t[:, hh, 0, :], in_=q8ni[:, hh, bsl])
                    nc.sync.dma_start(out=q8_t[:, hh, 1, :], in_=q8ri[:, bsl])
                nc.sync.dma_start(out=k8_t[:, :, 0, :], in_=knin[:, :, bsl])
                nc.sync.dma_start(out=k8_t[:, :, 1, :], in_=kpin[:, :, bsl])
                nc.sync.dma_start(
                    out=v_t,
                    in_=vin[bsl, :].rearrange("(kt p) v -> p kt v", p=128))
                if bb == 0:
                    nc.sync.dma_start(out=mask_t, in_=maskp[:, :])
                    nc.sync.dma_start(out=ones_col, in_=onec[:, :])
                    nc.sync.dma_start(out=ones_row, in_=oner[:, :])
                    nc.sync.dma_start(out=wop_t, in_=wop[:, :, :])

            wo_queue = []
            outs_map = {}

            def wo_chunk(bb, o_tt, t16, ch):
                tsl = slice(t16 * 128, (t16 + 1) * 128)
                acc = ps.tile([128, 512], F32, tag="mm", bufs=4, name="acc")
                for hh in range(2):
                    nc.tensor.matmul(acc, o_tt[:, hh, tsl],
                                     wop_t[:, hh, ch * 512:(ch + 1) * 512],
                                     start=(hh == 0), stop=(hh == 1))
                if ch == 0:
                    outs_map[(bb, t16)] = sb.tile([128, DIM], BF16, tag="outs",
                                                  bufs=2, name="outs")
                outs = outs_map[(bb, t16)]
                eng = nc.scalar.copy if ch == 1 else nc.vector.tensor_copy
                eng(outs[:, ch * 512:(ch + 1) * 512], acc)
                if ch % 2 == 1:
                    half = slice((ch - 1) * 512, (ch + 1) * 512)
                    nc.sync.dma_start(
                        out=out[bb * S + t16 * 128:bb * S + (t16 + 1) * 128, half],
                        in_=outs[:, half])
                    if ch == 3:
                        del outs_map[(bb, t16)]

            load_b(0)
            for b in range(B):
                q8_t, k8_t, v_t, o_t = btiles[b]

                def normalize1(pend):
                    hh, lacc_p, oacc_p, qsl_p = pend
                    inv = sb.tile([1, 512], F32R, tag="inv", bufs=2)
                    with nc.allow_low_precision(reason="fp32r rounding of softmax denom"):
                        nc.vector.reciprocal(inv, lacc_p)
                    return (hh, inv, oacc_p, qsl_p)

                def normalize2(pend):
                    hh, inv, oacc_p, qsl_p = pend
                    bc = ps.tile([128, 512], F32, tag="mm", bufs=4)
                    nc.tensor.matmul(bc, ones_row, inv, start=True, stop=True)
                    bcs = sb.tile([128, 512], BF16, tag="bcs", bufs=2)
                    nc.scalar.copy(bcs, bc)
                    with nc.allow_low_precision(reason="bf16 attn output"):
                        nc.vector.tensor_mul(o_t[:, hh, qsl_p], oacc_p, bcs)

                pend_box = [None]

                def attn_qt(qt):
                    for h in range(2):
                        qsl = slice(qt * 512, (qt + 1) * 512)
                        nkt = 4 * qt + 4
                        lacc = ps.tile([1, 512], F32, tag="row", bufs=2, name="lacc")
                        oacc = ps.tile([128, 512], F32, tag="pv", bufs=2, name="oacc")

                        lst = {"started": False, "pend": None}

                        def lacc_mm(src, off_p, w_p, last):
                            nc.tensor.matmul(lacc[:, off_p:512], ones_col,
                                             src[:, :w_p],
                                             start=(not lst["started"]), stop=last)
                            lst["started"] = True

                        def consume(prev_e):
                            et_p, off_p, w_p, kt_p = prev_e
                            last = (kt_p == nkt - 1)
                            j = kt_p - 4 * qt
                            if 0 <= j < 4:
                                # only the 128-col diagonal block needs masking
                                with nc.allow_low_precision(reason="bf16 probs"):
                                    nc.vector.tensor_mul(et_p[:, 0:128],
                                                         et_p[:, 0:128], mask_t)
                            if off_p == 0 and not last:
                                # pair full-width tiles: one denominator
                                # matmul per two tiles (shallow DVE add)
                                if lst["pend"] is None:
                                    lst["pend"] = et_p
                                else:
                                    es = sb.tile([128, 512], BF16, tag="es",
                                                 bufs=3)
                                    with nc.allow_low_precision(reason="bf16 denom"):
                                        nc.vector.tensor_add(es, lst["pend"], et_p)
                                    lst["pend"] = None
                                    lacc_mm(es, 0, 512, False)
                            else:
                                if lst["pend"] is not None:
                                    lacc_mm(lst["pend"], 0, 512, False)
                                    lst["pend"] = None
                                lacc_mm(et_p, off_p, w_p, last)
                            nc.tensor.matmul(oacc[:, off_p:512],
                                             v_t[:, kt_p, h * 128:(h + 1) * 128],
                                             et_p[:, :w_p],
                                             start=(kt_p == 0), stop=(kt_p == nkt - 1))

                        pend2 = []
                        for kt in range(nkt):
                            ksl = slice(kt * 128, (kt + 1) * 128)
                            j = kt - 4 * qt
                            # columns of this q-tile that can be unmasked:
                            off = 0 if j < 1 else 128 * j
                            w = 512 - off
                            qs2 = slice(qt * 512 + off, (qt + 1) * 512)
                            sc = ps.tile([128, 512], F32, tag="mm", bufs=4, name="sc")
                            nc.tensor.matmul(sc[:, :w], k8_t[:, h, :, ksl],
                                             q8_t[:, h, :, qs2],
                                             start=True, stop=True,
                                             perf_mode=mybir.MatmulPerfMode.DoubleRow)
                            if len(pend2) >= 3:
                                consume(pend2.pop(0))
                            et = sb.tile([128, 512], BF16, tag="exp", bufs=6)
                            nc.scalar.activation(et[:, :w], sc[:, :w], AF.Exp,
                                                 scale=SCALE)
                            pend2.append((et, off, w, kt))
                            if kt == 0:
                                if pend_box[0] is not None:
                                    pend_box[0] = normalize1(pend_box[0])
                            elif kt == 2 and pend_box[0] is not None:
                                normalize2(pend_box[0])
                                pend_box[0] = None
                            elif kt >= 3 and wo_queue:
                                wo_chunk(*wo_queue.pop(0))
                                if len(wo_queue) > 16:
                                    wo_chunk(*wo_queue.pop(0))
                        for e in pend2:
                            consume(e)
                        pend_box[0] = (h, lacc, oacc, qsl)
                        if h == 1:
                            wo_queue.extend((b, o_t, t16, ch) for t16 in
                                            range(qt * 4, qt * 4 + 4)
                                            for ch in range(4))

                attn_qt(0)
                attn_qt(1)
                if b == 0:
                    load_b(1)
                attn_qt(2)
                attn_qt(3)
                if pend_box[0] is not None:
                    normalize2(normalize1(pend_box[0]))
                    pend_box[0] = None
                if b == B - 1:
                    for e in wo_queue:
                        wo_chunk(*e)
                    wo_queue.clear()

    nc.compile()
    return nc


# --------------------------------------------------------------------------
# Host-side data prep
# --------------------------------------------------------------------------
def _pack(wT, ktiles):
    """(ktiles*128, M) -> (128, ktiles, M) with [p, k, m] = wT[k*128+p, m]."""
    K, M = wT.shape
    assert K == ktiles * 128
    return np.ascontiguousarray(
        wT.reshape(ktiles, 128, M).transpose(1, 0, 2)).astype(NPBF)


def _swap_pairs(a, axis):
    idx = np.arange(a.shape[axis])
    idx = idx.reshape(-1, 2)[:, ::-1].reshape(-1)
    return np.take(a, idx, axis=axis)


def _prep(inputs):
    x = np.asarray(inputs["x"], dtype=np.float32)
    f = np.asarray(inputs["freqs_cis"], dtype=np.float32)
    wq_a = np.asarray(inputs["wq_a"], dtype=np.float32)
    wq_b = np.asarray(inputs["wq_b"], dtype=np.float32)
    q_norm_w = np.asarray(inputs["q_norm_w"], dtype=np.float32)
    wkv_a = np.asarray(inputs["wkv_a"], dtype=np.float32)
    kv_norm_w = np.asarray(inputs["kv_norm_w"], dtype=np.float32)
    wkv_b = np.asarray(inputs["wkv_b"], dtype=np.float32)
    wo = np.asarray(inputs["wo"], dtype=np.float32)

    xT = np.ascontiguousarray(x.reshape(T, DIM).T).astype(NPBF)  # (DIM, T)

    cos = f[:, :, 0].T  # (32, S)
    sin = f[:, :, 1].T
    cosF = np.empty((DR, S), np.float32)
    sinF = np.empty((DR, S), np.float32)
    cosF[0::2] = cos
    cosF[1::2] = cos
    sinF[0::2] = -sin
    sinF[1::2] = sin

    wqaT = wq_a.T                       # (DIM, RQ)
    wkvaT = wkv_a.T                     # (DIM, RKV+DR)
    pe = wkvaT[:, RKV:RKV + DR]
    wqa_p = _pack(wqaT, 16)
    wkva_p = _pack(wkvaT[:, :RKV], 16)
    wkvap_p = _pack(np.concatenate([pe, _swap_pairs(pe, 1)], axis=1), 16)
    one_c = np.ones((128, 1), NPBF)

    wqbT = (wq_b * q_norm_w[None, :]).T       # (RQ, H*DQK)
    wkvbT = (wkv_b * kv_norm_w[None, :]).T    # (RKV, H*(DN+DV))
    woT = wo.T                                # (H*DV, DIM)
    kn_cols = np.concatenate(
        [wkvbT[:, h * (DN + DV):h * (DN + DV) + DN] for h in range(H)], axis=1)
    v_cols = np.concatenate(
        [wkvbT[:, h * (DN + DV) + DN:(h + 1) * (DN + DV)] for h in range(H)],
        axis=1)
    wkbf_p = _pack(kn_cols, 4)
    wvbf_p = _pack(v_cols, 4)
    # q ext cols: [nope_h x16][pe-pair x8][sw-pair x8]
    nope_cols = [wqbT[:, h * DQK:h * DQK + DN] for h in range(H)]
    pe_cols = [wqbT[:, h * DQK + DN:(h + 1) * DQK] for h in range(H)]
    sw_cols = [_swap_pairs(p, 1) for p in pe_cols]
    pe_pairs = [np.concatenate(pe_cols[2 * i:2 * i + 2], axis=1) for i in range(8)]
    sw_pairs = [np.concatenate(sw_cols[2 * i:2 * i + 2], axis=1) for i in range(8)]
    wqbe_p = _pack(np.concatenate(nope_cols + pe_pairs + sw_pairs, axis=1), 4)
    cosdF = np.concatenate([cosF, cosF], axis=0)   # (128, S)
    sindF = np.concatenate([sinF, sinF], axis=0)

    k1_maps = []
    for c in range(NCORES):
        t0 = c * TS
        srange = slice(t0 % S, t0 % S + TS)
        k1_maps.append({
            "xt": np.ascontiguousarray(xT[:, t0:t0 + TS]),
            "wqa": wqa_p, "wkva": wkva_p, "wkvap": wkvap_p,
            "wkbf": wkbf_p, "wvbf": wvbf_p, "wqbe": wqbe_p,
            "cosk": np.ascontiguousarray(cosF[:, srange]),
            "sink": np.ascontiguousarray(sinF[:, srange]),
            "cosd": np.ascontiguousarray(cosdF[:, srange]).astype(NPBF),
            "sind": np.ascontiguousarray(sindF[:, srange]).astype(NPBF),
            "onec": one_c,
            "oner": np.ones((1, 128), np.float32),
        })

    # mask block: [p, c] = 1 if c >= p (valid within the 128-col diag block)
    mask = (np.arange(128)[None, :] >= np.arange(128)[:, None]).astype(NPBF)

    k2_maps = []
    for c in range(NCORES):
        h0, h1 = 2 * c, 2 * c + 1
        worows = np.concatenate([woT[hh * DV:(hh + 1) * DV] for hh in (h0, h1)], axis=0)
        k2_maps.append({
            "wop": _pack(worows, 2),
            "maskp": mask,
            "onec": one_c,
            "oner": np.ones((1, 128), np.float32),
        })
    return k1_maps, k2_maps


def _get(name, builder):
    if name not in _CACHE:
        _CACHE[name] = builder()
    return _CACHE[name]


def _run(inputs, trace=False):
    k1_maps, k2_maps = _prep(inputs)
    nc1 = _get("k1", build_k1)
    r1 = run_bass_kernel_spmd(nc1, k1_maps, core_ids=list(range(NCORES)), trace=trace)

    q8n = np.concatenate([r1.results[c]["q8n"] for c in range(NCORES)], axis=2)
    q8r = np.concatenate([r1.results[c]["q8r"] for c in range(NCORES)], axis=2)
    knT = np.concatenate([r1.results[c]["knT"] for c in range(NCORES)], axis=1)
    vfull = np.concatenate([r1.results[c]["vout"] for c in range(NCORES)], axis=0)
    kpe = np.concatenate([r1.results[c]["kpe"] for c in range(NCORES)], axis=1)
    # pair-1 of the k8 tiles, prebuilt: [kp; 0] for h0, [0; kp] for h1
    kpin = np.zeros((128, 2, T), NPF8)
    kpin[0:DR, 0] = kpe
    kpin[DR:128, 1] = kpe
    for c, m in enumerate(k2_maps):
        m["q8ni"] = np.ascontiguousarray(q8n[:, 2 * c:2 * c + 2, :])
        m["q8ri"] = np.ascontiguousarray(q8r[:, c, :])
        m["kpin"] = kpin
        m["knin"] = np.ascontiguousarray(
            knT[c * 256:(c + 1) * 256].reshape(2, 128, T).transpose(1, 0, 2))
        m["vin"] = np.ascontiguousarray(vfull[:, c * 256:(c + 1) * 256])

    nc2 = _get("k2", build_k2)
    r2 = run_bass_kernel_spmd(nc2, k2_maps, core_ids=list(range(NCORES)), trace=trace)

    acc = r2.results[0]["out"].astype(np.float32)
    for c in range(1, NCORES):
        acc = acc + r2.results[c]["out"].astype(np.float32)
    return acc.reshape(B, S, DIM), (r1, r2)


def kernel(**inputs) -> np.ndarray:
    out, _ = _run(inputs)
    return out
